# revision 11
# baseline (speedup 1.0000x reference)
"""Trainium2 Bass kernel for ACLIP top-k patch masking.

Reference computation (per batch):
    cls, patches = split(image_features)            # [1,D], [P,D]  P=576
    sim = normalize(patches) @ normalize(text)      # [P]
    idx = sort(top_k(sim, 288).indices)             # [288]
    out = concat([cls, patches[idx]])               # [289, D]

Distribution: pure data parallel, batch 256 -> 32 per core x 8 cores.

Per-core algorithm (B=32 batches, P=576 patches, D=1024, K=288):
  - Load patch rows [128, 5, 1024] per batch (chunk 4 half-filled).
  - prod = X * text_bcast (text norm is a positive per-batch constant and
    cannot change the top-k ordering, so text is used unnormalized).
  - s[p] = sum_d prod[p, d]  (fused tensor_scalar reduce / ACT copy-accum)
  - n[p] = sum_d X[p, d]^2   (ACT Square with accum_out)
  - r = s * rsqrt(n)
  - rank[p] = #{q: r[q] > r[p]} exactly, via fused compare-reduce of each
    r-column against a partition-broadcast row of all 576 sims
    (DVE: tensor_scalar is_gt + accum; ACT: Sign(r_q - r_p) + accum).
  - keep = rank < 288; dest slot = cumsum(keep) via triangular matmul;
    dropped rows get dest=1e6 and are skipped by the indirect-DMA bounds
    check. Kept rows land in out rows [b*289+1, b*289+288] in spatial
    order (== sorted top-k indices). CLS rows written by a strided DMA.

Engine split is tuned so DVE / ACT / GPSIMD all stay near the ~317us/core
HBM roofline (113.5 MB/core at ~358 GB/s).
"""

import numpy as np

import concourse.bass as bass
import concourse.mybir as mybir
import concourse.tile as tile
from concourse import bacc
from concourse.bass import IndirectOffsetOnAxis
from concourse.masks import make_identity, make_upper_triangular

F32 = mybir.dt.float32
I32 = mybir.dt.int32

B_FULL = 256
N_CORES = 8
B_CORE = B_FULL // N_CORES
NUM_TOKENS = 577
P = 576          # patches per batch
D = 1024
K = 288          # kept patches
OUT_TOK = K + 1  # cls + kept
NCH = 5          # 128-row chunks per batch (4 full + 1 of 64)
LAST = P - 4 * 128  # rows in last chunk = 64
# Skip sentinel for dropped rows. Must be f32-exact, > any valid row index,
# and small enough that sentinel * D stays within int32 (the indirect DMA
# multiplies indices by the row stride).
BIG = 1.0e6

# Per-chunk engine assignment (index = chunk 0..4; chunk 4 is 64 rows).
MULT_ENGINE = ["dve", "dve", "dve", "gp", "gp"]
SRED_ENGINE = ["dve", "dve", "dve", "dve", "dve"]
RANK_ENGINE = ["act", "act", "act", "dve", "dve"]


def build(nc, b_core=B_CORE, img=None, txt=None, out=None):
    if img is None:
        img = nc.dram_tensor("image_features", [b_core, NUM_TOKENS, D], F32,
                             kind="ExternalInput").ap()
        txt = nc.dram_tensor("text_features", [b_core, D], F32,
                             kind="ExternalInput").ap()
        out = nc.dram_tensor("out", [b_core, OUT_TOK, D], F32,
                             kind="ExternalOutput").ap()

    out_flat = out.rearrange("b k d -> (b k) d")

    with tile.TileContext(nc) as tc:
        with (
            tc.tile_pool(name="consts", bufs=1) as consts,
            tc.tile_pool(name="x", bufs=3) as xpool,
            tc.tile_pool(name="prod", bufs=2) as prpool,
            tc.tile_pool(name="bcast", bufs=3) as bcpool,
            tc.tile_pool(name="small", bufs=4) as spool,
            tc.tile_pool(name="junk", bufs=2) as jpool,
            tc.tile_pool(name="psum", bufs=2, space="PSUM") as ppool,
        ):
            ident = consts.tile([128, 128], F32)
            make_identity(nc, ident[:])
            ltri = consts.tile([128, 128], F32)
            make_upper_triangular(nc, ltri[:], val=1.0, diag=True)
            ones_col = consts.tile([128, 1], F32)
            nc.vector.memset(ones_col[:], 1.0)

            # CLS passthrough for all batches (SBUF bounce).
            clsbuf = consts.tile([b_core, D], F32)
            nc.sync.dma_start(out=clsbuf[:], in_=img[:, 0, :])
            nc.sync.dma_start(out=out[:, 0, :], in_=clsbuf[:])

            for b in range(b_core):
                # ---- loads ----
                x = xpool.tile([128, NCH, D], F32, tag="x")
                nc.sync.dma_start(
                    out=x[:, 0:4, :],
                    in_=img[b, 1:513, :].rearrange("(c p) d -> p c d", p=128),
                )
                nc.sync.dma_start(out=x[0:LAST, 4, :], in_=img[b, 513:577, :])

                trow = spool.tile([1, D], F32, tag="trow")
                nc.sync.dma_start(out=trow[:], in_=txt[b : b + 1, :])
                txtb = bcpool.tile([128, D], F32, tag="txtb")
                nc.gpsimd.partition_broadcast(txtb[:], trow[:1, :], channels=128)

                # ---- prod = X * text, s = sum(prod), n = sum(X^2) ----
                S = spool.tile([128, NCH], F32, tag="S")
                N = spool.tile([128, NCH], F32, tag="N")
                nc.vector.memset(S[LAST:128, 4:5], 0.0)
                nc.vector.memset(N[LAST:128, 4:5], 1.0)
                prod = prpool.tile([128, NCH, D], F32, tag="prod")
                for c in range(NCH):
                    rows = 128 if c < 4 else LAST
                    eng = nc.vector if MULT_ENGINE[c] == "dve" else nc.gpsimd
                    eng.tensor_tensor(
                        out=prod[:rows, c, :], in0=x[:rows, c, :],
                        in1=txtb[:rows, :], op=mybir.AluOpType.mult,
                    )
                    if SRED_ENGINE[c] == "dve":
                        jv = jpool.tile([128, D], F32, tag="jv")
                        nc.vector.tensor_scalar(
                            out=jv[:rows, :], in0=prod[:rows, c, :],
                            scalar1=1.0, scalar2=0.0,
                            op0=mybir.AluOpType.mult, op1=mybir.AluOpType.add,
                            accum_out=S[:rows, c : c + 1],
                        )
                    else:
                        jv = jpool.tile([128, D], F32, tag="jvact")
                        nc.scalar.activation(
                            out=jv[:rows, :], in_=prod[:rows, c, :],
                            func=mybir.ActivationFunctionType.Copy,
                            accum_out=S[:rows, c : c + 1],
                        )
                    js = jpool.tile([128, D], F32, tag="js")
                    nc.scalar.activation(
                        out=js[:rows, :], in_=x[:rows, c, :],
                        func=mybir.ActivationFunctionType.Square,
                        accum_out=N[:rows, c : c + 1],
                    )

                # ---- r = s / sqrt(n) ----
                SQ = spool.tile([128, NCH], F32, tag="SQ")
                nc.scalar.sqrt(SQ[:], N[:])
                REC = spool.tile([128, NCH], F32, tag="REC")
                nc.vector.reciprocal(REC[:], SQ[:])
                R = spool.tile([128, NCH], F32, tag="R")
                nc.vector.tensor_tensor(
                    out=R[:], in0=S[:], in1=REC[:], op=mybir.AluOpType.mult
                )
                # garbage rows of the half chunk must never rank into top-K
                nc.vector.memset(R[LAST:128, 4:5], -1e30)
                # negated sims: bias operand for the ACT Sign rank variant
                NEGR = spool.tile([128, NCH], F32, tag="NEGR")
                nc.vector.tensor_scalar(
                    out=NEGR[:], in0=R[:], scalar1=-1.0, scalar2=None,
                    op0=mybir.AluOpType.mult,
                )

                # ---- all-sims row: transpose each r-column, broadcast ----
                rbc = bcpool.tile([128, P], F32, tag="rbc")
                for c in range(NCH):
                    w = 128 if c < 4 else LAST
                    rpsum = ppool.tile([1, 128], F32, tag="rpsum")
                    nc.tensor.transpose(rpsum[:], R[:, c : c + 1], ident[:])
                    rrow = spool.tile([1, 128], F32, tag="rrow")
                    nc.scalar.copy(rrow[:], rpsum[:])
                    nc.gpsimd.partition_broadcast(
                        rbc[:, c * 128 : c * 128 + w], rrow[:1, 0:w],
                        channels=128,
                    )

                # ---- exact ranks ----
                RANK = spool.tile([128, NCH], F32, tag="RANK")
                for c in range(NCH):
                    rows = 128 if c < 4 else LAST
                    if RANK_ENGINE[c] == "dve":
                        # rank = sum_q [r_q > r_p]
                        jr = jpool.tile([128, P], F32, tag="jr")
                        nc.vector.tensor_scalar(
                            out=jr[:rows, :], in0=rbc[:rows, :],
                            scalar1=R[:rows, c : c + 1], scalar2=0.0,
                            op0=mybir.AluOpType.is_gt,
                            op1=mybir.AluOpType.add,
                            accum_out=RANK[:rows, c : c + 1],
                        )
                    else:
                        # sum_q sign(r_q - r_p) = 2*rank - 575 (no ties)
                        jr = jpool.tile([128, P], F32, tag="jract")
                        nc.scalar.activation(
                            out=jr[:rows, :], in_=rbc[:rows, :],
                            func=mybir.ActivationFunctionType.Sign,
                            bias=NEGR[:rows, c : c + 1], scale=1.0,
                            accum_out=RANK[:rows, c : c + 1],
                        )
                        # rescale sign-sum to rank
                        nc.vector.tensor_scalar(
                            out=RANK[:rows, c : c + 1],
                            in0=RANK[:rows, c : c + 1],
                            scalar1=0.5, scalar2=float(P - 1) / 2.0,
                            op0=mybir.AluOpType.mult,
                            op1=mybir.AluOpType.add,
                        )
                nc.vector.memset(RANK[LAST:128, 4:5], 1e9)

                # ---- keep mask and destination slots ----
                mask = spool.tile([128, NCH], F32, tag="mask")
                nc.vector.tensor_scalar(
                    out=mask[:], in0=RANK[:], scalar1=float(K), scalar2=None,
                    op0=mybir.AluOpType.is_lt,
                )
                cpsum = ppool.tile([128, NCH], F32, tag="cpsum")
                nc.tensor.matmul(cpsum[:], lhsT=ltri[:], rhs=mask[:],
                                 start=True, stop=True)
                cum = spool.tile([128, NCH], F32, tag="cum")
                nc.scalar.copy(cum[:], cpsum[:])

                # exclusive per-chunk offsets from the chunk totals
                tpsum = ppool.tile([1, NCH], F32, tag="tpsum")
                nc.tensor.matmul(tpsum[:], lhsT=ones_col[:], rhs=mask[:],
                                 start=True, stop=True)
                tot = spool.tile([1, NCH], F32, tag="tot")
                nc.scalar.copy(tot[:], tpsum[:])
                oinc = spool.tile([1, NCH], F32, tag="oinc")
                nc.vector.tensor_tensor_scan(
                    out=oinc[:], data0=tot[:], data1=tot[:],
                    initial=0.0, op0=mybir.AluOpType.add,
                    op1=mybir.AluOpType.bypass,
                )
                offx = spool.tile([1, NCH], F32, tag="offx")
                nc.vector.tensor_tensor(
                    out=offx[:], in0=oinc[:], in1=tot[:],
                    op=mybir.AluOpType.subtract,
                )
                obc = spool.tile([128, NCH], F32, tag="obc")
                nc.gpsimd.partition_broadcast(obc[:], offx[:1, :], channels=128)

                G = spool.tile([128, NCH], F32, tag="G")
                nc.vector.tensor_tensor(out=G[:], in0=cum[:], in1=obc[:],
                                        op=mybir.AluOpType.add)
                LE = spool.tile([128, NCH], F32, tag="LE")
                nc.vector.tensor_scalar(
                    out=LE[:], in0=G[:], scalar1=float(K), scalar2=None,
                    op0=mybir.AluOpType.is_le,
                )
                VAL = spool.tile([128, NCH], F32, tag="VAL")
                nc.vector.tensor_tensor(out=VAL[:], in0=mask[:], in1=LE[:],
                                        op=mybir.AluOpType.mult)
                # dest = valid ? G + b*289 : BIG
                A = spool.tile([128, NCH], F32, tag="A")
                nc.vector.tensor_scalar(
                    out=A[:], in0=G[:], scalar1=float(b * OUT_TOK) - BIG,
                    scalar2=None, op0=mybir.AluOpType.add,
                )
                M2 = spool.tile([128, NCH], F32, tag="M2")
                nc.vector.tensor_tensor(out=M2[:], in0=A[:], in1=VAL[:],
                                        op=mybir.AluOpType.mult)
                DF = spool.tile([128, NCH], F32, tag="DF")
                nc.vector.tensor_scalar(
                    out=DF[:], in0=M2[:], scalar1=BIG, scalar2=None,
                    op0=mybir.AluOpType.add,
                )
                desti = spool.tile([128, NCH], I32, tag="desti")
                nc.vector.tensor_copy(out=desti[:], in_=DF[:])

                # ---- scatter kept rows (one offset column per chunk) ----
                for c in range(NCH):
                    rows = 128 if c < 4 else LAST
                    nc.gpsimd.indirect_dma_start(
                        out=out_flat[:, :],
                        out_offset=IndirectOffsetOnAxis(
                            ap=desti[0:rows, c : c + 1], axis=0
                        ),
                        in_=x[0:rows, c, :],
                        in_offset=None,
                        bounds_check=b * OUT_TOK + K,
                        oob_is_err=False,
                    )
    return nc


_CACHED = {}


def _get_nc():
    if "nc" not in _CACHED:
        nc = bacc.Bacc("TRN2", target_bir_lowering=False)
        build(nc)
        nc.compile()
        _CACHED["nc"] = nc
    return _CACHED["nc"]


LAST_RESULT = None


def kernel(image_features, text_features):
    global LAST_RESULT
    from concourse.bass_utils import run_bass_kernel_spmd

    img = np.ascontiguousarray(np.asarray(image_features, dtype=np.float32))
    txt = np.ascontiguousarray(np.asarray(text_features, dtype=np.float32))
    assert img.shape == (B_FULL, NUM_TOKENS, D)
    assert txt.shape == (B_FULL, D)

    nc = _get_nc()
    in_maps = [
        {
            "image_features": img[i * B_CORE : (i + 1) * B_CORE],
            "text_features": txt[i * B_CORE : (i + 1) * B_CORE],
        }
        for i in range(N_CORES)
    ]
    res = run_bass_kernel_spmd(nc, in_maps, core_ids=list(range(N_CORES)))
    LAST_RESULT = res
    return np.concatenate([res.results[i]["out"] for i in range(N_CORES)], axis=0)


# revision 14
# speedup vs baseline: 1.0455x; 1.0455x over previous
"""Trainium2 Bass kernel for ACLIP top-k patch masking.

Reference computation (per batch):
    cls, patches = split(image_features)            # [1,D], [P,D]  P=576
    sim = normalize(patches) @ normalize(text)      # [P]
    idx = sort(top_k(sim, 288).indices)             # [288]
    out = concat([cls, patches[idx]])               # [289, D]

Distribution: pure data parallel, batch 256 -> 32 per core x 8 cores.

Per-core algorithm (B=32 batches, P=576 patches, D=1024, K=288):
  - Load patch rows [128, 5, 1024] per batch (chunk 4 half-filled).
  - prod = X * text_bcast (text norm is a positive per-batch constant and
    cannot change the top-k ordering, so text is used unnormalized).
  - s[p] = sum_d prod[p, d], n[p] = sum_d X[p, d]^2, r = s * rsqrt(n).
  - rank[p] = #{q: r[q] > r[p]} exactly, by comparing each r-column
    against a row of all 576 sims (built by PE transpose + PE ones-matmul
    broadcast into PSUM). DVE chunks: tensor_scalar is_gt + accum.
    ACT chunks: Sign(r_p - r_q) + accum gives 575 - 2*rank, so the keep
    test rank < 288 becomes signsum >= 0.
  - dest slot = cumsum(keep) via triangular matmul. Kept rows are written
    by an indirect scatter DMA to rows [b*289+1 ...]; dropped rows get
    dest=1e6 and tie-overflow slots (cumsum > K) exceed the DMA bounds
    check, so both are skipped. CLS rows go by a strided DMA.

Work is split so DVE / ACT / GPSIMD all stay near the HBM roofline
(113.5 MB/core at ~358 GB/s ~= 317 us/core).
"""

import numpy as np

import concourse.bass as bass
import concourse.mybir as mybir
import concourse.tile as tile
from concourse import bacc
from concourse.bass import IndirectOffsetOnAxis
from concourse.masks import make_identity, make_upper_triangular

F32 = mybir.dt.float32
I32 = mybir.dt.int32

B_FULL = 256
N_CORES = 8
B_CORE = B_FULL // N_CORES
NUM_TOKENS = 577
P = 576          # patches per batch
D = 1024
K = 288          # kept patches
OUT_TOK = K + 1  # cls + kept
NCH = 5          # 128-row chunks per batch (4 full + 1 of 64)
LAST = P - 4 * 128  # rows in last chunk = 64
# Skip sentinel for dropped rows. Must be f32-exact, > any valid row index,
# and small enough that sentinel * D stays within int32 (the indirect DMA
# multiplies indices by the row stride).
BIG = 1.0e6

# chunk 4's multiply runs on GPSIMD; ranks for chunks 0-1 run on ACT (Sign)
MULT_GP = (4,)
RANK_ACT = (0, 1)


def build(nc, b_core=B_CORE, img=None, txt=None, out=None):
    if img is None:
        img = nc.dram_tensor("image_features", [b_core, NUM_TOKENS, D], F32,
                             kind="ExternalInput").ap()
        txt = nc.dram_tensor("text_features", [b_core, D], F32,
                             kind="ExternalInput").ap()
        out = nc.dram_tensor("out", [b_core, OUT_TOK, D], F32,
                             kind="ExternalOutput").ap()

    out_flat = out.rearrange("b k d -> (b k) d")

    with tile.TileContext(nc) as tc:
        with (
            tc.tile_pool(name="consts", bufs=1) as consts,
            tc.tile_pool(name="x", bufs=4) as xpool,
            tc.tile_pool(name="prod", bufs=2) as prpool,
            tc.tile_pool(name="bcast", bufs=3) as bcpool,
            tc.tile_pool(name="small", bufs=6) as spool,
            tc.tile_pool(name="junk", bufs=3) as jpool,
            tc.tile_pool(name="ps_row", bufs=2, space="PSUM") as pprow,
            tc.tile_pool(name="ps_rbc", bufs=2, space="PSUM") as pprbc,
            tc.tile_pool(name="ps_cum", bufs=2, space="PSUM") as ppcum,
        ):
            ident = consts.tile([128, 128], F32)
            make_identity(nc, ident[:])
            ltri = consts.tile([128, 128], F32)
            make_upper_triangular(nc, ltri[:], val=1.0, diag=True)
            ones_col = consts.tile([128, 1], F32)
            nc.vector.memset(ones_col[:], 1.0)
            ones_row = consts.tile([1, 128], F32)
            nc.vector.memset(ones_row[:], 1.0)

            # CLS passthrough for all batches (SBUF bounce).
            clsbuf = consts.tile([b_core, D], F32)
            nc.sync.dma_start(out=clsbuf[:], in_=img[:, 0, :])
            nc.sync.dma_start(out=out[:, 0, :], in_=clsbuf[:])

            for b in range(b_core):
                # ---- loads ----
                x = xpool.tile([128, NCH, D], F32, tag="x")
                nc.sync.dma_start(
                    out=x[:, 0:4, :],
                    in_=img[b, 1:513, :].rearrange("(c p) d -> p c d", p=128),
                )
                nc.sync.dma_start(out=x[0:LAST, 4, :], in_=img[b, 513:577, :])

                trow = spool.tile([1, D], F32, tag="trow")
                nc.sync.dma_start(out=trow[:], in_=txt[b : b + 1, :])
                txtb = bcpool.tile([128, D], F32, tag="txtb")
                nc.gpsimd.partition_broadcast(txtb[:], trow[:1, :], channels=128)

                # ---- prod = X * text; s = sum(prod); n = sum(X^2) ----
                S = spool.tile([128, NCH], F32, tag="S")
                N = spool.tile([128, NCH], F32, tag="N")
                nc.vector.memset(S[LAST:128, 4:5], 0.0)
                nc.vector.memset(N[LAST:128, 4:5], 1.0)

                prod = prpool.tile([128, 4, D], F32, tag="prod")
                nc.vector.tensor_tensor(
                    out=prod[:, :, :], in0=x[:, 0:4, :],
                    in1=txtb[:, None, :].to_broadcast([128, 4, D]),
                    op=mybir.AluOpType.mult,
                )
                nc.vector.tensor_reduce(
                    out=S[:, 0:4], in_=prod[:, :, :],
                    axis=mybir.AxisListType.X, op=mybir.AluOpType.add,
                )
                prod4 = prpool.tile([128, D], F32, tag="prod4")
                nc.gpsimd.tensor_tensor(
                    out=prod4[0:LAST, :], in0=x[0:LAST, 4, :],
                    in1=txtb[0:LAST, :], op=mybir.AluOpType.mult,
                )
                ja = jpool.tile([128, D], F32, tag="ja")
                nc.scalar.activation(
                    out=ja[0:LAST, :], in_=prod4[0:LAST, :],
                    func=mybir.ActivationFunctionType.Copy,
                    accum_out=S[0:LAST, 4:5],
                )
                for c in range(NCH):
                    rows = 128 if c < 4 else LAST
                    js = jpool.tile([128, D], F32, tag="ja")
                    nc.scalar.activation(
                        out=js[:rows, :], in_=x[:rows, c, :],
                        func=mybir.ActivationFunctionType.Square,
                        accum_out=N[:rows, c : c + 1],
                    )

                # ---- r = s / sqrt(n) ----
                SQ = spool.tile([128, NCH], F32, tag="SQ")
                nc.scalar.sqrt(SQ[:], N[:])
                REC = spool.tile([128, NCH], F32, tag="REC")
                nc.vector.reciprocal(REC[:], SQ[:])
                R = spool.tile([128, NCH], F32, tag="R")
                nc.vector.tensor_tensor(
                    out=R[:], in0=S[:], in1=REC[:], op=mybir.AluOpType.mult
                )
                # garbage rows of the half chunk must never rank into top-K
                nc.vector.memset(R[LAST:128, 4:5], -1e30)

                # ---- all-sims row in PSUM: transpose cols, ones-matmul ----
                rbcps = pprbc.tile([128, P], F32, tag="rbcps")
                for c in range(NCH):
                    w = 128 if c < 4 else LAST
                    rpsum = pprow.tile([1, 128], F32, tag="rpsum")
                    nc.tensor.transpose(rpsum[:], R[:, c : c + 1], ident[:])
                    rrow = spool.tile([1, 128], F32, tag="rrow")
                    nc.scalar.copy(rrow[:], rpsum[:])
                    nc.tensor.matmul(
                        rbcps[:, c * 128 : c * 128 + w],
                        lhsT=ones_row[:],
                        rhs=rrow[:1, 0:w],
                        start=True, stop=True,
                    )

                # ---- exact ranks ----
                # DVE chunks: RANK = #{q: r_q > r_p}; keep iff RANK < K.
                # ACT chunks: RANK = sum_q sign(r_p - r_q) = 575 - 2*rank;
                #             keep iff RANK >= 0.
                RANK = spool.tile([128, NCH], F32, tag="RANK")
                for c in range(NCH):
                    rows = 128 if c < 4 else LAST
                    if c in RANK_ACT:
                        jr = jpool.tile([128, P], F32, tag="jract")
                        nc.scalar.activation(
                            out=jr[:rows, :], in_=rbcps[:rows, :],
                            func=mybir.ActivationFunctionType.Sign,
                            bias=R[:rows, c : c + 1], scale=-1.0,
                            accum_out=RANK[:rows, c : c + 1],
                        )
                    else:
                        jr = jpool.tile([128, P], F32, tag="jrdve")
                        nc.vector.tensor_scalar(
                            out=jr[:rows, :], in0=rbcps[:rows, :],
                            scalar1=R[:rows, c : c + 1], scalar2=0.0,
                            op0=mybir.AluOpType.is_gt,
                            op1=mybir.AluOpType.add,
                            accum_out=RANK[:rows, c : c + 1],
                        )
                nc.vector.memset(RANK[LAST:128, 4:5], 1e9)

                # ---- keep mask and destination slots ----
                mask = spool.tile([128, NCH], F32, tag="mask")
                na = len(RANK_ACT)  # ACT chunks are 0..na-1 (contiguous)
                if na:
                    nc.vector.tensor_scalar(
                        out=mask[:, 0:na], in0=RANK[:, 0:na],
                        scalar1=0.0, scalar2=None,
                        op0=mybir.AluOpType.is_ge,
                    )
                nc.vector.tensor_scalar(
                    out=mask[:, na:NCH], in0=RANK[:, na:NCH],
                    scalar1=float(K), scalar2=None,
                    op0=mybir.AluOpType.is_lt,
                )
                cpsum = ppcum.tile([128, NCH], F32, tag="cpsum")
                nc.tensor.matmul(cpsum[:], lhsT=ltri[:], rhs=mask[:],
                                 start=True, stop=True)
                cum = spool.tile([128, NCH], F32, tag="cum")
                nc.scalar.copy(cum[:], cpsum[:])

                # exclusive per-chunk offsets from the chunk totals
                tpsum = pprow.tile([1, NCH], F32, tag="rpsum")
                nc.tensor.matmul(tpsum[:], lhsT=ones_col[:], rhs=mask[:],
                                 start=True, stop=True)
                tot = spool.tile([1, NCH], F32, tag="tot")
                nc.scalar.copy(tot[:], tpsum[:])
                oinc = spool.tile([1, NCH], F32, tag="oinc")
                nc.vector.tensor_tensor_scan(
                    out=oinc[:], data0=tot[:], data1=tot[:],
                    initial=0.0, op0=mybir.AluOpType.add,
                    op1=mybir.AluOpType.bypass,
                )
                offx = spool.tile([1, NCH], F32, tag="offx")
                nc.vector.tensor_tensor(
                    out=offx[:], in0=oinc[:], in1=tot[:],
                    op=mybir.AluOpType.subtract,
                )
                obc = spool.tile([128, NCH], F32, tag="obc")
                nc.gpsimd.partition_broadcast(obc[:], offx[:1, :], channels=128)

                # dest = mask ? cumsum + b*289 : BIG; slots beyond K (tie
                # overflow) exceed the scatter bounds check and are dropped.
                G = spool.tile([128, NCH], F32, tag="G")
                nc.vector.tensor_tensor(out=G[:], in0=cum[:], in1=obc[:],
                                        op=mybir.AluOpType.add)
                W = spool.tile([128, NCH], F32, tag="W")
                nc.vector.tensor_scalar(
                    out=W[:], in0=mask[:], scalar1=-BIG, scalar2=BIG + float(b * OUT_TOK),
                    op0=mybir.AluOpType.mult, op1=mybir.AluOpType.add,
                )
                DF = spool.tile([128, NCH], F32, tag="DF")
                nc.vector.tensor_tensor(out=DF[:], in0=G[:], in1=W[:],
                                        op=mybir.AluOpType.add)
                desti = spool.tile([128, NCH], I32, tag="desti")
                nc.vector.tensor_copy(out=desti[:], in_=DF[:])

                # ---- scatter kept rows (one offset column per chunk) ----
                for c in range(NCH):
                    rows = 128 if c < 4 else LAST
                    nc.gpsimd.indirect_dma_start(
                        out=out_flat[:, :],
                        out_offset=IndirectOffsetOnAxis(
                            ap=desti[0:rows, c : c + 1], axis=0
                        ),
                        in_=x[0:rows, c, :],
                        in_offset=None,
                        bounds_check=b * OUT_TOK + K,
                        oob_is_err=False,
                    )
    return nc


_CACHED = {}


def _get_nc():
    if "nc" not in _CACHED:
        nc = bacc.Bacc("TRN2", target_bir_lowering=False)
        build(nc)
        nc.compile()
        _CACHED["nc"] = nc
    return _CACHED["nc"]


LAST_RESULT = None


def kernel(image_features, text_features):
    global LAST_RESULT
    from concourse.bass_utils import run_bass_kernel_spmd

    img = np.ascontiguousarray(np.asarray(image_features, dtype=np.float32))
    txt = np.ascontiguousarray(np.asarray(text_features, dtype=np.float32))
    assert img.shape == (B_FULL, NUM_TOKENS, D)
    assert txt.shape == (B_FULL, D)

    nc = _get_nc()
    in_maps = [
        {
            "image_features": img[i * B_CORE : (i + 1) * B_CORE],
            "text_features": txt[i * B_CORE : (i + 1) * B_CORE],
        }
        for i in range(N_CORES)
    ]
    res = run_bass_kernel_spmd(nc, in_maps, core_ids=list(range(N_CORES)))
    LAST_RESULT = res
    return np.concatenate([res.results[i]["out"] for i in range(N_CORES)], axis=0)
